# revision 10
# baseline (speedup 1.0000x reference)
"""Bahdanau (additive) attention kernel for 8x Trainium2 NeuronCores.

Reference computation (per problem nn_Attn_3075196583966):
    qp = q @ WQ.T + bQ                    [N, D]
    kp = k @ WK.T + bK                    [M, D]
    vp = v @ WV.T + bV                    [M, D]
    score[n,m] = sum_d Ww[d] * tanh(qp[n,d] + kp[m,d]) + bw
    score = where(mask==1, score, -1e6)
    w = softmax(score, axis=1)
    out = w @ vp                          [N, D]

Sharding: N (queries) split across 8 cores (32 each); k/v/weights replicated.
Each core is fully independent (no collectives).

Per-core implementation notes:
  - kp/qp are computed TRANSPOSED ([d, m] / [d, n], d on partitions) so the
    scalar engine can evaluate tanh(kpT + qpT[:, n]) in one ACTIVATE per
    (d-chunk, query) using the per-partition bias operand.
  - The weighted reduction over d is a matmul with a per-query stationary
    matrix Wbig[:, dc, n, :] (Ww chunk in column n, zeros elsewhere), so all
    32 queries' scores accumulate into a single PSUM bank [32, 512] per
    m-half -- no PSUM gather needed (DMA cannot read PSUM).
  - bw cancels in softmax and is dropped.  bV is added via a rank-1 matmul
    (ones x bV) into the context PSUM accumulation (softmax weights sum to 1).
  - exp() uses the ACT accum_out to produce row sums in the same pass.
"""

import sys

import numpy as np

if "/opt/trn_rl_repo" not in sys.path:
    sys.path.insert(0, "/opt/trn_rl_repo")

N, M, D = 256, 1024, 512
NCORES = 8
NLOC = N // NCORES  # 32 queries per core
P = 128
NEC = D // P  # 4 contraction chunks
NDC = D // P  # 4 feature chunks
NMB = M // P  # 8 key blocks
MH = 2  # m halves (PSUM bank = 512 fp32)

_CACHE = {}


def _build_nc(debug=()):
    if debug is True:
        debug = ("qpT", "kpT", "scores", "masked", "expw", "sums", "vp", "wT", "Wbig", "kT")
    from contextlib import ExitStack

    import concourse.bacc as bacc
    import concourse.mybir as mybir
    import concourse.tile as tile
    from concourse.masks import make_identity

    f32 = mybir.dt.float32
    f16 = mybir.dt.float16
    i32 = mybir.dt.int32
    AF = mybir.ActivationFunctionType
    ALU = mybir.AluOpType
    AX = mybir.AxisListType

    nc = bacc.Bacc("TRN2", target_bir_lowering=False)

    q = nc.dram_tensor("q", [NLOC, D], f32, kind="ExternalInput")
    k = nc.dram_tensor("k", [M, D], f32, kind="ExternalInput")
    v = nc.dram_tensor("v", [M, D], f32, kind="ExternalInput")
    mask = nc.dram_tensor("mask", [NLOC, M], i32, kind="ExternalInput")
    WQ = nc.dram_tensor("WQ", [D, D], f32, kind="ExternalInput")
    bQ = nc.dram_tensor("bQ", [D], f32, kind="ExternalInput")
    WK = nc.dram_tensor("WK", [D, D], f32, kind="ExternalInput")
    bK = nc.dram_tensor("bK", [D], f32, kind="ExternalInput")
    WV = nc.dram_tensor("WV", [D, D], f32, kind="ExternalInput")
    bV = nc.dram_tensor("bV", [D], f32, kind="ExternalInput")
    Ww = nc.dram_tensor("Ww", [1, D], f32, kind="ExternalInput")
    out = nc.dram_tensor("out", [NLOC, D], f32, kind="ExternalOutput")
    dbg_specs = {
        "qpT": ([P, NDC, NLOC], f32), "kpT": ([P, NDC, M], f16),
        "scores": ([NLOC, M], f32), "masked": ([NLOC, M], f32),
        "expw": ([NLOC, M], f16), "sums": ([NLOC, 1], f32),
        "vp": ([P, NMB, D], f16), "wT": ([P, NMB, NLOC], f16),
        "Wbig": ([P, NDC, NLOC, NLOC], f16), "kT": ([P, NEC, M], f16),
    }
    dbg = {}
    for name in debug:
        shp, dt_ = dbg_specs[name]
        dbg[name] = nc.dram_tensor(f"dbg_{name}", shp, dt_, kind="ExternalOutput")

    k_r = k.rearrange("(mb p) e -> p mb e", p=P)
    v_r = v.rearrange("(mb p) e -> p mb e", p=P)
    WQ_r = WQ.rearrange("(dc p) e -> p dc e", p=P)
    WK_r = WK.rearrange("(dc p) e -> p dc e", p=P)
    WV_r = WV.rearrange("(dc p) e -> p dc e", p=P)

    with tile.TileContext(nc) as tc, ExitStack() as ctx:
        sb = ctx.enter_context(tc.tile_pool(name="sb", bufs=1))
        tpool = ctx.enter_context(tc.tile_pool(name="tpool", bufs=4))
        tp = ctx.enter_context(tc.tile_pool(name="tp", bufs=3, space="PSUM"))
        pp = ctx.enter_context(tc.tile_pool(name="pp", bufs=3, space="PSUM"))
        scp = ctx.enter_context(tc.tile_pool(name="scp", bufs=2, space="PSUM"))

        dma = nc.sync.dma_start

        def sbt(shape, dtype, tag):
            return sb.tile(shape, dtype, tag=tag, name=tag)

        # persistent SBUF tensors
        id128 = sbt([P, P], f32, "id128")
        id32f = sbt([NLOC, NLOC], f32, "id32f")
        id32h = sbt([NLOC, NLOC], f16, "id32h")
        idmask = sbt([P, NLOC, NLOC], f16, "idmask")
        q_sb = sbt([NLOC, D], f32, "q_sb")
        qT_sb = sbt([P, NEC, NLOC], f16, "qT_sb")
        WQ_sb = sbt([P, NDC, D], f32, "WQ_sb")
        WK_sb = sbt([P, NDC, D], f32, "WK_sb")
        WV_sb = sbt([P, NDC, D], f32, "WV_sb")
        WQT_sb = sbt([P, NEC, D], f16, "WQT_sb")
        WKT_sb = sbt([P, NEC, D], f16, "WKT_sb")
        WVT_sb = sbt([P, NEC, D], f16, "WVT_sb")
        k_sb = sbt([P, NMB, D], f32, "k_sb")
        v_sb = sbt([P, NMB, D], f32, "v_sb")
        kT_sb = sbt([P, NEC, M], f16, "kT_sb")
        vT_sb = sbt([P, NEC, M], f16, "vT_sb")
        kpT_sb = sbt([P, NDC, M], f16, "kpT_sb")
        vp_sb = sbt([P, NMB, D], f16, "vp_sb")
        qpT_sb = sbt([P, NDC, NLOC], f32, "qpT_sb")
        bQ4 = sbt([P, NDC], f32, "bQ4")
        bK4 = sbt([P, NDC], f32, "bK4")
        bQK = sbt([P, NDC], f32, "bQK")
        w4_sb = sbt([P, NDC], f32, "w4_sb")
        Wbig = sbt([P, NDC, NLOC, NLOC], f16, "Wbig")
        bV_bc = sbt([NLOC, D], f32, "bV_bc")
        mask_sb = sbt([NLOC, M], i32, "mask_sb")
        maskf = sbt([NLOC, M], f32, "maskf")
        penalty = sbt([NLOC, M], f32, "penalty")
        scores_sb = sbt([NLOC, M], f32, "scores_sb")
        masked = sbt([NLOC, M], f32, "masked")
        rowmax = sbt([NLOC, 1], f32, "rowmax")
        negmax = sbt([NLOC, 1], f32, "negmax")
        expw_h = sbt([NLOC, M], f16, "expw_h")
        sums = sbt([NLOC, 1], f32, "sums")
        rsum = sbt([NLOC, 1], f32, "rsum")
        wT_sb = sbt([P, NMB, NLOC], f16, "wT_sb")
        out_sb = sbt([NLOC, D], f32, "out_sb")

        # ---- phase 0: identities / constants
        make_identity(nc, id128)
        make_identity(nc, id32f)
        make_identity(nc, id32h)
        nc.gpsimd.memset(idmask, 0.0)
        nc.gpsimd.affine_select(
            out=idmask,
            in_=idmask,
            compare_op=ALU.not_equal,
            fill=1.0,
            base=0,
            pattern=[[1, NLOC], [-1, NLOC]],
            channel_multiplier=0,
        )

        # ---- phase 1: q path (qpT = WQ @ q^T + (bQ + bK), [d, n])
        dma(out=q_sb, in_=q[:])
        dma(out=WQ_sb, in_=WQ_r)
        dma(out=bQ4, in_=bQ.rearrange("(c p) -> p c", p=P))
        dma(out=bK4, in_=bK.rearrange("(c p) -> p c", p=P))
        nc.vector.tensor_add(bQK, bQ4, bK4)

        for ec in range(NEC):
            ps = tp.tile([P, P], f32, tag="tp")
            nc.tensor.transpose(ps[:, :NLOC], q_sb[:, ec * P : (ec + 1) * P], id32f)
            nc.vector.tensor_copy(out=qT_sb[:, ec, :], in_=ps[:, :NLOC])
        for dc in range(NDC):
            for ec in range(NEC):
                ps = tp.tile([P, P], f32, tag="tp")
                nc.tensor.transpose(ps, WQ_sb[:, dc, ec * P : (ec + 1) * P], id128)
                nc.vector.tensor_copy(out=WQT_sb[:, ec, dc * P : (dc + 1) * P], in_=ps)
        for dc in range(NDC):
            ps = pp.tile([P, D], f32, tag="pp")
            for ec in range(NEC):
                nc.tensor.matmul(
                    ps[:, :NLOC],
                    WQT_sb[:, ec, dc * P : (dc + 1) * P],
                    qT_sb[:, ec, :],
                    start=(ec == 0),
                    stop=(ec == NEC - 1),
                )
            nc.vector.tensor_scalar_add(qpT_sb[:, dc, :], ps[:, :NLOC], bQK[:, dc : dc + 1])

        # ---- phase 2: k path (kpT = WK @ k^T, [d, m])
        for mb in range(NMB):
            dma(out=k_sb[:, mb, :], in_=k_r[:, mb, :])
        dma(out=WK_sb, in_=WK_r)
        for mb in range(NMB):
            for ec in range(NEC):
                ps = tp.tile([P, P], f32, tag="tp")
                nc.tensor.transpose(ps, k_sb[:, mb, ec * P : (ec + 1) * P], id128)
                nc.vector.tensor_copy(out=kT_sb[:, ec, mb * P : (mb + 1) * P], in_=ps)
        for dc in range(NDC):
            for ec in range(NEC):
                ps = tp.tile([P, P], f32, tag="tp")
                nc.tensor.transpose(ps, WK_sb[:, dc, ec * P : (ec + 1) * P], id128)
                nc.vector.tensor_copy(out=WKT_sb[:, ec, dc * P : (dc + 1) * P], in_=ps)
        for dc in range(NDC):
            for mh in range(MH):
                ps = pp.tile([P, D], f32, tag="pp")
                for ec in range(NEC):
                    nc.tensor.matmul(
                        ps,
                        WKT_sb[:, ec, dc * P : (dc + 1) * P],
                        kT_sb[:, ec, mh * D : (mh + 1) * D],
                        start=(ec == 0),
                        stop=(ec == NEC - 1),
                    )
                nc.vector.tensor_copy(out=kpT_sb[:, dc, mh * D : (mh + 1) * D], in_=ps)

        # score weight columns: Wbig[p, dc, n, j] = (n == j) * Ww[dc*128 + p]
        dma(out=w4_sb, in_=Ww.rearrange("o (c p) -> p (o c)", p=P))
        for dc in range(NDC):
            nc.vector.tensor_scalar_mul(Wbig[:, dc], idmask, w4_sb[:, dc : dc + 1])

        # ---- phase 3: main loop -- tanh on ScalarE + weighted-reduce on PE
        from concourse.tile_rust import add_dep_helper

        score_ps = [scp.tile([NLOC, D], f32, tag="sc", name=f"score_ps{mh}") for mh in range(MH)]
        # The start=True matmul clears has_written for the whole bank, so the
        # 128 accumulating matmuls per bank MUST execute after it.  The Tile
        # scheduler does not order same-tile matmul writes, so chain them
        # explicitly (same-engine deps lower to pure program order -- free).
        prev_mm = [None] * MH
        for dc in range(NDC):
            for n in range(NLOC):
                t = tpool.tile([P, M], f16, tag="t")
                nc.scalar.activation(
                    t,
                    kpT_sb[:, dc, :],
                    AF.Tanh,
                    bias=qpT_sb[:, dc, n : n + 1],
                )
                for mh in range(MH):
                    mm = nc.tensor.matmul(
                        score_ps[mh],
                        Wbig[:, dc, n, :],
                        t[:, mh * D : (mh + 1) * D],
                        start=(dc == 0 and n == 0),
                        stop=(dc == NDC - 1 and n == NLOC - 1),
                    )
                    if prev_mm[mh] is not None:
                        add_dep_helper(
                            mm.ins,
                            prev_mm[mh].ins,
                            reason="score accumulation order (start clears bank)",
                        )
                    prev_mm[mh] = mm
        for mh in range(MH):
            nc.vector.tensor_copy(out=scores_sb[:, mh * D : (mh + 1) * D], in_=score_ps[mh])

        # ---- phase 4: v path (vp = v @ WV.T, [m, d]); low priority, fills PE idle
        for mb in range(NMB):
            dma(out=v_sb[:, mb, :], in_=v_r[:, mb, :])
        dma(out=WV_sb, in_=WV_r)
        dma(out=bV_bc, in_=bV[None, :].to_broadcast((NLOC, D)))
        for mb in range(NMB):
            for ec in range(NEC):
                ps = tp.tile([P, P], f32, tag="tp")
                nc.tensor.transpose(ps, v_sb[:, mb, ec * P : (ec + 1) * P], id128)
                nc.vector.tensor_copy(out=vT_sb[:, ec, mb * P : (mb + 1) * P], in_=ps)
        for dc in range(NDC):
            for ec in range(NEC):
                ps = tp.tile([P, P], f32, tag="tp")
                nc.tensor.transpose(ps, WV_sb[:, dc, ec * P : (ec + 1) * P], id128)
                nc.vector.tensor_copy(out=WVT_sb[:, ec, dc * P : (dc + 1) * P], in_=ps)
        for mb in range(NMB):
            ps = pp.tile([P, D], f32, tag="pp")
            for ec in range(NEC):
                nc.tensor.matmul(
                    ps,
                    vT_sb[:, ec, mb * P : (mb + 1) * P],
                    WVT_sb[:, ec, :],
                    start=(ec == 0),
                    stop=(ec == NEC - 1),
                )
            nc.vector.tensor_copy(out=vp_sb[:, mb, :], in_=ps)

        # ---- phase 5: mask + softmax (weights left unnormalized; divide at end)
        dma(out=mask_sb, in_=mask[:])
        nc.vector.tensor_copy(out=maskf, in_=mask_sb)
        nc.vector.tensor_scalar(
            out=penalty,
            in0=maskf,
            scalar1=1.0e6,
            scalar2=-1.0e6,
            op0=ALU.mult,
            op1=ALU.add,
        )
        nc.vector.tensor_add(masked, scores_sb, penalty)
        nc.vector.reduce_max(rowmax, masked, axis=AX.X)
        nc.vector.tensor_scalar_mul(negmax, rowmax, -1.0)
        nc.scalar.activation(
            expw_h,
            masked,
            AF.Exp,
            bias=negmax[:, 0:1],
            accum_out=sums,
        )
        nc.vector.reciprocal(rsum, sums)

        # ---- phase 6: context = (expw @ vp + ones*bV) * rsum
        for mb in range(NMB):
            ps = tp.tile([P, P], f16, tag="tp")
            nc.tensor.transpose(
                ps[:, :NLOC], expw_h[:, mb * P : (mb + 1) * P], id32h
            )
            nc.vector.tensor_copy(out=wT_sb[:, mb, :], in_=ps[:, :NLOC])
        ctx_ps = pp.tile([NLOC, D], f32, tag="pp")
        prev_ctx = None
        for mb in range(NMB):
            mm = nc.tensor.matmul(
                ctx_ps,
                wT_sb[:, mb, :],
                vp_sb[:, mb, :],
                start=(mb == 0),
                stop=(mb == NMB - 1),
            )
            if prev_ctx is not None:
                add_dep_helper(
                    mm.ins, prev_ctx.ins, reason="ctx accumulation order"
                )
            prev_ctx = mm
        nc.vector.tensor_scalar_mul(out_sb, ctx_ps, rsum[:, 0:1])
        nc.vector.tensor_add(out_sb, out_sb, bV_bc)
        dma(out=out[:], in_=out_sb)
        dbg_srcs = {
            "qpT": qpT_sb, "kpT": kpT_sb, "scores": scores_sb, "masked": masked,
            "expw": expw_h, "sums": sums, "vp": vp_sb, "wT": wT_sb,
            "Wbig": Wbig, "kT": kT_sb,
        }
        for name in debug:
            dma(out=dbg[name][:], in_=dbg_srcs[name])

    nc.finalize()
    return nc


def _get_nc():
    if "nc" not in _CACHE:
        _CACHE["nc"] = _build_nc()
    return _CACHE["nc"]


def _run(inputs, trace=False, trace_kwargs=None):
    from concourse.bass_utils import run_bass_kernel_spmd

    nc = _get_nc()

    def f32(x):
        return np.ascontiguousarray(np.asarray(x, dtype=np.float32))

    q = f32(inputs["q"])
    mask = np.ascontiguousarray(np.asarray(inputs["mask"], dtype=np.int32))
    shared = {
        "k": f32(inputs["k"]),
        "v": f32(inputs["v"]),
        "WQ": f32(inputs["WQ"]),
        "bQ": f32(inputs["bQ"]),
        "WK": f32(inputs["WK"]),
        "bK": f32(inputs["bK"]),
        "WV": f32(inputs["WV"]),
        "bV": f32(inputs["bV"]),
        "Ww": f32(inputs["Ww"]),
    }
    in_maps = []
    for c in range(NCORES):
        im = dict(shared)
        im["q"] = np.ascontiguousarray(q[c * NLOC : (c + 1) * NLOC])
        im["mask"] = np.ascontiguousarray(mask[c * NLOC : (c + 1) * NLOC])
        in_maps.append(im)

    res = run_bass_kernel_spmd(
        nc,
        in_maps,
        core_ids=list(range(NCORES)),
        trace=trace,
        **(trace_kwargs or {}),
    )
    full = np.concatenate([r["out"] for r in res.results], axis=0)
    return full.astype(np.float32), res


def kernel(**inputs):
    return _run(inputs)[0]
